# revision 13
# baseline (speedup 1.0000x reference)
"""Trainium2 Bass kernel for nn_BaseLUTLayer (soft-LUT layer), node-sharded.

Math: out[b,o] = sum_k lut[o,k] * prod_j (bit_j(k) ? x[b,m(o,j)] : 1-x[b,m(o,j)])

Walsh/multilinear form: with u = 2x-1 and ahat = lut @ W / 64 (host-side
Walsh transform over the 6-bit index), out = sum_S ahat_S * prod_{j in S} u_j.
Evaluated as a 6-level multilinear tree T_j[s] = T_{j+1}[s] + u_j*T_{j+1}[s+2^j].
The (static per run) mapping gather runs on HOST; the device streams
pre-gathered u rows via plain DMA.

Per core (node-sharded 8 ways): nodes [256c, 256(c+1)) as 2 chunks of 128
nodes-on-partitions. L1 (fused MACs vs per-node ahat scalars) + L2 + L3-mul
run once per chunk at full 1024 batch, IN PLACE in the t1 buffer:
    t1[16:32] *= u4 ; t1[0:16] += t1[16:32]   (L2)
    t1[8:16]  *= u3                            (L3 mul)
From L3-add down, work splits into 2 batch halves of 512: t3 is built in
PSUM by PE (t3[s] = t1[s] + t1[s+8]), and L4-L6 adds accumulate in place in
the same PSUM banks; DVE supplies pr4/pn2/pn1 muls (PSUM reads, fp32 1x).
L1 is split DVE tensor_scalar (4x mode) / ScalarE activations by NTS.
PSUM drained by ScalarE copy, then DMA.
"""

import numpy as np
import ml_dtypes

import concourse.bass as bass
import concourse.mybir as mybir
from concourse import bacc
from concourse import tile
from concourse.masks import make_identity
from concourse.bass_utils import run_bass_kernel_spmd

P = 128
IN = 1024
OUT = 2048
NB = 6
B_FULL = 1024
N_CORES = 8
NODES_PER_CORE = OUT // N_CORES  # 256
NCHUNK = NODES_PER_CORE // P     # 2
NHALF = 2
BH = B_FULL // NHALF             # 512
F32 = mybir.dt.float32
BF16 = mybir.dt.bfloat16

# DVE tensor_scalar L1 slices per chunk (rest go to ScalarE activations).
NTS = [10, 8]


def _mult():
    return mybir.AluOpType.mult


def _add():
    return mybir.AluOpType.add


def build_program():
    nc = bacc.Bacc("TRN2", target_bir_lowering=False, debug=False)

    gz = nc.dram_tensor("gz", [P, NCHUNK, NB, B_FULL], BF16, kind="ExternalInput").ap()
    lutg = nc.dram_tensor("lutg", [P, NCHUNK, 64], F32, kind="ExternalInput").ap()
    outs = nc.dram_tensor("outs", [P, NCHUNK, NHALF, BH], F32, kind="ExternalOutput").ap()

    with tile.TileContext(nc) as tc:
        with (
            tc.tile_pool(name="consts", bufs=1) as consts,
            tc.tile_pool(name="zpool", bufs=2) as zpool,
            tc.tile_pool(name="t1pool", bufs=2) as t1pool,
            tc.tile_pool(name="spool", bufs=2) as spool,
            tc.tile_pool(name="opool", bufs=2) as opool,
            tc.tile_pool(name="psum", bufs=1, space="PSUM") as psum,
        ):
            lutg_sb = consts.tile([P, NCHUNK, 64], F32)
            nc.sync.dma_start(lutg_sb, lutg)
            ident = consts.tile([P, P], BF16)
            make_identity(nc, ident)

            zs = {}
            t1s = {}
            state = {}

            def gather(c, part=None):
                # plain DMA of host-pre-gathered u rows (2KB each)
                if part is None or part == 0:
                    z = zpool.tile([P, NB, B_FULL], BF16, tag="z")
                    zs[c] = z
                z = zs[c]
                if part is None:
                    s0, s1 = 0, NB
                else:
                    s0, s1 = [(0, 1), (1, 3), (3, NB)][part]
                nc.sync.dma_start(z[:, s0:s1, :], gz[:, c, s0:s1, :])

            def u(c, s):
                return zs[c][:, s, :]  # [P, 1024]

            def uh(c, s, h):
                return zs[c][:, s, h * BH : (h + 1) * BH]

            def _splits(c):
                nts = NTS[c]
                dve_ks = list(range(16, 16 + min(nts, 16)))
                if nts > 16:
                    dve_ks += list(range(0, nts - 16))
                sc_hi = list(range(16 + min(nts, 16), 32))
                sc_lo = list(range(max(0, nts - 16), 16))
                sc_ks = sc_hi + sc_lo[8:][::-1] + sc_lo[:8][::-1]
                return dve_ks, sc_ks

            def scalar_l1(c):
                t1 = t1pool.tile([P, 32, B_FULL], BF16, tag="t1")
                t1s[c] = t1
                r5 = u(c, 0)
                _, sc_ks = _splits(c)
                for k in sc_ks:
                    nc.scalar.activation(
                        t1[:, k, :],
                        r5,
                        mybir.ActivationFunctionType.Identity,
                        bias=lutg_sb[:, c, k : k + 1],
                        scale=lutg_sb[:, c, 32 + k : 33 + k],
                    )

            def dve_l1(c):
                t1 = t1s[c]
                r5 = u(c, 0)
                dve_ks, _ = _splits(c)
                for k in dve_ks:
                    nc.vector.tensor_scalar(
                        out=t1[:, k, :],
                        in0=r5,
                        scalar1=lutg_sb[:, c, 32 + k : 33 + k],
                        scalar2=lutg_sb[:, c, k : k + 1],
                        op0=_mult(),
                        op1=_add(),
                    )

            def dve_l23(c):
                # in-place L2 + L3 mul at full 1024 width inside t1
                t1 = t1s[c]
                nc.vector.tensor_mul(
                    t1[:, 16:32, :],
                    u(c, 1)[:, None, :].broadcast_to([P, 16, B_FULL]),
                    t1[:, 16:32, :],
                )
                nc.vector.tensor_add(
                    t1[:, 0:16, :].rearrange("p a b -> p (a b)"),
                    t1[:, 16:32, :].rearrange("p a b -> p (a b)"),
                    t1[:, 0:16, :].rearrange("p a b -> p (a b)"),
                )
                nc.vector.tensor_mul(
                    t1[:, 8:16, :],
                    u(c, 2)[:, None, :].broadcast_to([P, 8, B_FULL]),
                    t1[:, 8:16, :],
                )

            def dve_half_pe(c, h):
                # From L3-add down on one batch half: t3 built in PSUM on PE,
                # pr4/pn2/pn1 muls on GpSimd (PSUM reads dodge DVE's SBUF
                # streams), adds accumulate in place on PE.
                t1 = t1s[c]
                th = t1[:, :, h * BH : (h + 1) * BH]
                # t3 hi half on DVE in SBUF (bf16); pr4 = u2 * t3[4:8]
                t3h = spool.tile([P, 4, BH], BF16, tag="t3h")
                nc.vector.tensor_add(t3h[:], th[:, 4:8, :], th[:, 12:16, :])
                pr4 = spool.tile([P, 4, BH], BF16, tag="pr4")
                nc.vector.tensor_mul(
                    pr4,
                    uh(c, 3, h)[:, None, :].broadcast_to([P, 4, BH]),
                    t3h[:],
                )
                acc = psum.tile([P, 4 * BH], F32, tag="accA")
                accv = acc[:].rearrange("p (a b) -> p a b", b=BH)
                for q in range(3, -1, -1):
                    sl = slice(q * BH, (q + 1) * BH)
                    nc.tensor.matmul(
                        acc[:, sl], ident, th[:, q, :], start=True, stop=False
                    )
                    nc.tensor.matmul(
                        acc[:, sl], ident, th[:, q + 8, :], start=False, stop=False
                    )
                    nc.tensor.matmul(
                        acc[:, sl], ident, pr4[:, q, :], start=False, stop=(q >= 2)
                    )
                # L5: pn2 = u1 * acc[2:4] ; acc[0:2] += pn2
                pn2 = spool.tile([P, 2, BH], BF16, tag="pn2")
                nc.vector.tensor_mul(
                    pn2,
                    uh(c, 4, h)[:, None, :].broadcast_to([P, 2, BH]),
                    accv[:, 2:4, :],
                )
                nc.tensor.matmul(
                    acc[:, BH : 2 * BH], ident, pn2[:, 1, :], start=False, stop=True
                )
                nc.tensor.matmul(
                    acc[:, 0:BH], ident, pn2[:, 0, :], start=False, stop=False
                )
                # L6: pn1 = u0 * acc[1:2] ; acc[0:1] += pn1
                pn1 = spool.tile([P, 1, BH], BF16, tag="pn1")
                nc.vector.tensor_mul(
                    pn1,
                    uh(c, 5, h)[:, None, :].broadcast_to([P, 1, BH]),
                    accv[:, 1:2, :],
                )
                nc.tensor.matmul(
                    acc[:, 0:BH], ident, pn1[:, 0, :], start=False, stop=True
                )
                state[("acc", c, h)] = acc

            def final(c, h):
                acc = state.pop(("acc", c, h))
                ot = opool.tile([P, BH], F32, tag="ot")
                nc.scalar.copy(ot[:], acc[:, 0:BH])
                nc.sync.dma_start(outs[:, c, h, :], ot[:])

            # ---- schedule ----
            gather(0, part=0)
            gather(0, part=1)
            gather(0, part=2)
            gather(1)
            scalar_l1(0)
            dve_l1(0)
            dve_l23(0)
            scalar_l1(1)
            dve_half_pe(0, 0)
            final(0, 0)
            dve_half_pe(0, 1)
            final(0, 1)
            dve_l1(1)
            dve_l23(1)
            dve_half_pe(1, 0)
            final(1, 0)
            dve_half_pe(1, 1)
            final(1, 1)

    nc.compile()
    return nc


_CACHE: dict = {}


def _program():
    if "nc" not in _CACHE:
        _CACHE["nc"] = build_program()
    return _CACHE["nc"]


def _walsh_matrix():
    k = np.arange(64)
    bits = ((k[:, None] >> np.arange(6)[None, :]) & 1) * 2 - 1  # [64, 6]
    s = np.arange(64)
    smask = ((s[:, None] >> np.arange(6)[None, :]) & 1).astype(bool)
    W = np.ones((64, 64), dtype=np.float64)
    for j in range(6):
        W *= np.where(smask[None, :, j], bits[:, None, j], 1)
    return W


def make_inputs(x, lut_table, mapping):
    x = np.ascontiguousarray(x, dtype=np.float32)
    lut_table = np.ascontiguousarray(lut_table, dtype=np.float32)
    mapping = np.asarray(mapping)

    uT = (2.0 * x.T - 1.0).astype(ml_dtypes.bfloat16)  # [i, b]
    ahat = (lut_table.astype(np.float64) @ _walsh_matrix() / 64.0).astype(np.float32)

    in_maps = []
    for core in range(N_CORES):
        mp = mapping[core * NODES_PER_CORE : (core + 1) * NODES_PER_CORE]
        mp3 = mp.reshape(NCHUNK, P, NB)
        # gz[p, c, s, b] = u[b, mp3[c, p, 5-s]]  (slot s = wire 5-s)
        idx = mp3[:, :, ::-1].transpose(1, 0, 2)  # [P, NCHUNK, NB]
        gz_arr = np.ascontiguousarray(uT[idx])    # [P, NCHUNK, NB, B_FULL]

        lut3 = ahat[core * NODES_PER_CORE : (core + 1) * NODES_PER_CORE]
        lutg_arr = np.ascontiguousarray(
            lut3.reshape(NCHUNK, P, 64).transpose(1, 0, 2)
        )

        in_maps.append({"gz": gz_arr, "lutg": lutg_arr})
    return in_maps


def assemble_output(results):
    out = np.empty((B_FULL, OUT), dtype=np.float32)
    for core in range(N_CORES):
        arr = results[core]["outs"]  # [o_p, c, h, b']
        blk = arr.transpose(2, 3, 1, 0).reshape(B_FULL, NODES_PER_CORE)
        out[:, core * NODES_PER_CORE : (core + 1) * NODES_PER_CORE] = blk
    return out


def kernel_with_results(x, lut_table, mapping, **kwargs):
    nc = _program()
    in_maps = make_inputs(x, lut_table, mapping)
    res = run_bass_kernel_spmd(nc, in_maps, core_ids=list(range(N_CORES)), **kwargs)
    return assemble_output(res.results), res


def kernel(x, lut_table, mapping):
    out, _ = kernel_with_results(x, lut_table, mapping)
    return out


if __name__ == "__main__":
    rng = np.random.default_rng(0)
    x = rng.random((B_FULL, IN), dtype=np.float32)
    lut = rng.standard_normal((OUT, 64), dtype=np.float32)
    mp = rng.integers(0, IN, (OUT, NB), dtype=np.int32)
    out = kernel(x, lut, mp)
    print(out.shape, out.dtype)


# revision 17
# speedup vs baseline: 1.1895x; 1.1895x over previous
"""Trainium2 Bass kernel for nn_BaseLUTLayer (soft-LUT layer), node-sharded.

Math: out[b,o] = sum_k lut[o,k] * prod_j (bit_j(k) ? x[b,m(o,j)] : 1-x[b,m(o,j)])

Walsh/multilinear form: with u = 2x-1 and ahat = lut @ W / 64 (host-side
Walsh transform over the 6-bit index), out = sum_S ahat_S * prod_{j in S} u_j.
Evaluated as a 6-level multilinear tree T_j[s] = T_{j+1}[s] + u_j*T_{j+1}[s+2^j].
The (static per run) mapping gather runs on HOST; the device streams
pre-gathered u rows via plain DMA.

Per core (node-sharded 8 ways): nodes [256c, 256(c+1)) as 2 chunks of 128
nodes-on-partitions. L1 (fused MACs vs per-node ahat scalars) + L2 + L3-mul
run once per chunk at full 1024 batch, IN PLACE in the t1 buffer:
    t1[16:32] *= u4 ; t1[0:16] += t1[16:32]   (L2)
    t1[8:16]  *= u3                            (L3 mul)
From L3-add down, work splits into 2 batch halves of 512: t3 is built in
PSUM by PE (t3[s] = t1[s] + t1[s+8]), and L4-L6 adds accumulate in place in
the same PSUM banks; DVE supplies pr4/pn2/pn1 muls (PSUM reads, fp32 1x).
L1 is split DVE tensor_scalar (4x mode) / ScalarE activations by NTS.
PSUM drained by ScalarE copy, then DMA.
"""

import numpy as np
import ml_dtypes

import concourse.bass as bass
import concourse.mybir as mybir
from concourse import bacc
from concourse import tile
from concourse.masks import make_identity
from concourse.bass_utils import run_bass_kernel_spmd

P = 128
IN = 1024
OUT = 2048
NB = 6
B_FULL = 1024
N_CORES = 8
NODES_PER_CORE = OUT // N_CORES  # 256
NCHUNK = NODES_PER_CORE // P     # 2
NHALF = 2
BH = B_FULL // NHALF             # 512
F32 = mybir.dt.float32
BF16 = mybir.dt.bfloat16

# DVE tensor_scalar L1 slices per chunk (rest go to ScalarE activations).
NTS = [13, 11]


def _mult():
    return mybir.AluOpType.mult


def _add():
    return mybir.AluOpType.add


def build_program():
    nc = bacc.Bacc("TRN2", target_bir_lowering=False, debug=False)

    gz = nc.dram_tensor("gz", [P, NCHUNK, NB, B_FULL], BF16, kind="ExternalInput").ap()
    lutg = nc.dram_tensor("lutg", [P, NCHUNK, 64], F32, kind="ExternalInput").ap()
    outs = nc.dram_tensor("outs", [P, NCHUNK, NHALF, BH], F32, kind="ExternalOutput").ap()

    with tile.TileContext(nc) as tc:
        with (
            tc.tile_pool(name="consts", bufs=1) as consts,
            tc.tile_pool(name="zpool", bufs=2) as zpool,
            tc.tile_pool(name="t1pool", bufs=2) as t1pool,
            tc.tile_pool(name="spool", bufs=2) as spool,
            tc.tile_pool(name="opool", bufs=2) as opool,
            tc.tile_pool(name="psum", bufs=2, space="PSUM") as psum,
        ):
            lutg_sb = consts.tile([P, NCHUNK, 64], F32)
            nc.sync.dma_start(lutg_sb, lutg)
            ident = consts.tile([P, P], BF16)
            make_identity(nc, ident)

            zs = {}
            t1s = {}
            state = {}

            def gather(c, part=None):
                # plain DMA of host-pre-gathered u rows (2KB each)
                if part is None or part == 0:
                    z = zpool.tile([P, NB, B_FULL], BF16, tag="z")
                    zs[c] = z
                z = zs[c]
                if part is None:
                    s0, s1 = 0, NB
                else:
                    s0, s1 = [(0, 1), (1, 3), (3, NB)][part]
                nc.sync.dma_start(z[:, s0:s1, :], gz[:, c, s0:s1, :])

            def u(c, s):
                return zs[c][:, s, :]  # [P, 1024]

            def uh(c, s, h):
                return zs[c][:, s, h * BH : (h + 1) * BH]

            def _splits(c):
                nts = NTS[c]
                dve_ks = list(range(16, 16 + min(nts, 16)))
                if nts > 16:
                    dve_ks += list(range(0, nts - 16))
                sc_hi = list(range(16 + min(nts, 16), 32))
                sc_lo = list(range(max(0, nts - 16), 16))
                sc_ks = sc_hi + sc_lo[8:][::-1] + sc_lo[:8][::-1]
                return dve_ks, sc_ks

            def scalar_l1(c):
                t1 = t1pool.tile([P, 32, B_FULL], BF16, tag="t1")
                t1s[c] = t1
                r5 = u(c, 0)
                _, sc_ks = _splits(c)
                for k in sc_ks:
                    nc.scalar.activation(
                        t1[:, k, :],
                        r5,
                        mybir.ActivationFunctionType.Identity,
                        bias=lutg_sb[:, c, k : k + 1],
                        scale=lutg_sb[:, c, 32 + k : 33 + k],
                    )

            def dve_l1(c):
                t1 = t1s[c]
                r5 = u(c, 0)
                dve_ks, _ = _splits(c)
                for k in dve_ks:
                    nc.vector.tensor_scalar(
                        out=t1[:, k, :],
                        in0=r5,
                        scalar1=lutg_sb[:, c, 32 + k : 33 + k],
                        scalar2=lutg_sb[:, c, k : k + 1],
                        op0=_mult(),
                        op1=_add(),
                    )

            def dve_l23(c):
                # in-place L2 + L3 mul at full 1024 width inside t1
                t1 = t1s[c]
                nc.vector.tensor_mul(
                    t1[:, 16:32, :],
                    u(c, 1)[:, None, :].broadcast_to([P, 16, B_FULL]),
                    t1[:, 16:32, :],
                )
                nc.vector.tensor_add(
                    t1[:, 0:16, :].rearrange("p a b -> p (a b)"),
                    t1[:, 16:32, :].rearrange("p a b -> p (a b)"),
                    t1[:, 0:16, :].rearrange("p a b -> p (a b)"),
                )
                nc.vector.tensor_mul(
                    t1[:, 8:16, :],
                    u(c, 2)[:, None, :].broadcast_to([P, 8, B_FULL]),
                    t1[:, 8:16, :],
                )

            def dve_half_pe(c, h):
                # From L3-add down on one batch half: t3 built in PSUM on PE,
                # pr4/pn2/pn1 muls on GpSimd (PSUM reads dodge DVE's SBUF
                # streams), adds accumulate in place on PE.
                t1 = t1s[c]
                th = t1[:, :, h * BH : (h + 1) * BH]
                # t3 hi half on DVE in SBUF (bf16); pr4 = u2 * t3[4:8]
                t3h = spool.tile([P, 4, BH], BF16, tag="t3h")
                nc.vector.tensor_add(t3h[:], th[:, 4:8, :], th[:, 12:16, :])
                pr4 = spool.tile([P, 4, BH], BF16, tag="pr4")
                nc.vector.tensor_mul(
                    pr4,
                    uh(c, 3, h)[:, None, :].broadcast_to([P, 4, BH]),
                    t3h[:],
                )
                if c == NCHUNK - 1 and h == NHALF - 1:
                    # last half: all-DVE tail (no PE ping-pong at the end)
                    t3l = spool.tile([P, 4, BH], BF16, tag="t3l")
                    nc.vector.tensor_add(t3l[:], th[:, 0:4, :], th[:, 8:12, :])
                    t4 = spool.tile([P, 4, BH], BF16, tag="t4d")
                    nc.vector.tensor_add(
                        t4[:].rearrange("p a b -> p (a b)"),
                        t3l[:].rearrange("p a b -> p (a b)"),
                        pr4[:].rearrange("p a b -> p (a b)"),
                    )
                    pn2 = spool.tile([P, 2, BH], BF16, tag="pn2")
                    nc.vector.tensor_mul(
                        pn2,
                        uh(c, 4, h)[:, None, :].broadcast_to([P, 2, BH]),
                        t4[:, 2:4, :],
                    )
                    t5 = spool.tile([P, 2, BH], BF16, tag="t5d")
                    nc.vector.tensor_add(t5, pn2, t4[:, 0:2, :])
                    pn1 = spool.tile([P, 1, BH], BF16, tag="pn1")
                    nc.vector.tensor_mul(
                        pn1,
                        uh(c, 5, h)[:, None, :].broadcast_to([P, 1, BH]),
                        t5[:, 1:2, :],
                    )
                    t6 = opool.tile([P, BH], F32, tag="t6d")
                    nc.vector.tensor_add(t6, pn1[:, 0, :], t5[:, 0, :])
                    state[("t6", c, h)] = t6
                    return
                acc = psum.tile([P, 4 * BH], F32, tag="accA")
                accv = acc[:].rearrange("p (a b) -> p a b", b=BH)
                for q in range(3, -1, -1):
                    sl = slice(q * BH, (q + 1) * BH)
                    nc.tensor.matmul(
                        acc[:, sl], ident, th[:, q, :], start=True, stop=False
                    )
                    nc.tensor.matmul(
                        acc[:, sl], ident, th[:, q + 8, :], start=False, stop=False
                    )
                    nc.tensor.matmul(
                        acc[:, sl], ident, pr4[:, q, :], start=False, stop=(q >= 2)
                    )
                # L5: pn2 = u1 * acc[2:4] ; acc[0:2] += pn2
                pn2 = spool.tile([P, 2, BH], BF16, tag="pn2")
                nc.vector.tensor_mul(
                    pn2,
                    uh(c, 4, h)[:, None, :].broadcast_to([P, 2, BH]),
                    accv[:, 2:4, :],
                )
                nc.tensor.matmul(
                    acc[:, BH : 2 * BH], ident, pn2[:, 1, :], start=False, stop=True
                )
                nc.tensor.matmul(
                    acc[:, 0:BH], ident, pn2[:, 0, :], start=False, stop=False
                )
                # L6: pn1 = u0 * acc[1:2] ; acc[0:1] += pn1
                pn1 = spool.tile([P, 1, BH], BF16, tag="pn1")
                nc.vector.tensor_mul(
                    pn1,
                    uh(c, 5, h)[:, None, :].broadcast_to([P, 1, BH]),
                    accv[:, 1:2, :],
                )
                nc.tensor.matmul(
                    acc[:, 0:BH], ident, pn1[:, 0, :], start=False, stop=True
                )
                state[("acc", c, h)] = acc

            def final(c, h):
                if ("t6", c, h) in state:
                    nc.sync.dma_start(outs[:, c, h, :], state.pop(("t6", c, h))[:])
                    return
                acc = state.pop(("acc", c, h))
                ot = opool.tile([P, BH], F32, tag="ot")
                nc.scalar.copy(ot[:], acc[:, 0:BH])
                nc.sync.dma_start(outs[:, c, h, :], ot[:])

            # ---- schedule ----
            gather(0, part=0)
            gather(0, part=1)
            gather(0, part=2)
            gather(1)
            scalar_l1(0)
            dve_l1(0)
            dve_l23(0)
            scalar_l1(1)
            dve_half_pe(0, 0)
            dve_l1(1)
            final(0, 0)
            dve_half_pe(0, 1)
            dve_l23(1)
            final(0, 1)
            dve_half_pe(1, 0)
            final(1, 0)
            dve_half_pe(1, 1)
            final(1, 1)

    nc.compile()
    return nc


_CACHE: dict = {}


def _program():
    if "nc" not in _CACHE:
        _CACHE["nc"] = build_program()
    return _CACHE["nc"]


def _walsh_matrix():
    k = np.arange(64)
    bits = ((k[:, None] >> np.arange(6)[None, :]) & 1) * 2 - 1  # [64, 6]
    s = np.arange(64)
    smask = ((s[:, None] >> np.arange(6)[None, :]) & 1).astype(bool)
    W = np.ones((64, 64), dtype=np.float64)
    for j in range(6):
        W *= np.where(smask[None, :, j], bits[:, None, j], 1)
    return W


def make_inputs(x, lut_table, mapping):
    x = np.ascontiguousarray(x, dtype=np.float32)
    lut_table = np.ascontiguousarray(lut_table, dtype=np.float32)
    mapping = np.asarray(mapping)

    uT = (2.0 * x.T - 1.0).astype(ml_dtypes.bfloat16)  # [i, b]
    ahat = (lut_table.astype(np.float64) @ _walsh_matrix() / 64.0).astype(np.float32)

    in_maps = []
    for core in range(N_CORES):
        mp = mapping[core * NODES_PER_CORE : (core + 1) * NODES_PER_CORE]
        mp3 = mp.reshape(NCHUNK, P, NB)
        # gz[p, c, s, b] = u[b, mp3[c, p, 5-s]]  (slot s = wire 5-s)
        idx = mp3[:, :, ::-1].transpose(1, 0, 2)  # [P, NCHUNK, NB]
        gz_arr = np.ascontiguousarray(uT[idx])    # [P, NCHUNK, NB, B_FULL]

        lut3 = ahat[core * NODES_PER_CORE : (core + 1) * NODES_PER_CORE]
        lutg_arr = np.ascontiguousarray(
            lut3.reshape(NCHUNK, P, 64).transpose(1, 0, 2)
        )

        in_maps.append({"gz": gz_arr, "lutg": lutg_arr})
    return in_maps


def assemble_output(results):
    out = np.empty((B_FULL, OUT), dtype=np.float32)
    for core in range(N_CORES):
        arr = results[core]["outs"]  # [o_p, c, h, b']
        blk = arr.transpose(2, 3, 1, 0).reshape(B_FULL, NODES_PER_CORE)
        out[:, core * NODES_PER_CORE : (core + 1) * NODES_PER_CORE] = blk
    return out


def kernel_with_results(x, lut_table, mapping, **kwargs):
    nc = _program()
    in_maps = make_inputs(x, lut_table, mapping)
    res = run_bass_kernel_spmd(nc, in_maps, core_ids=list(range(N_CORES)), **kwargs)
    return assemble_output(res.results), res


def kernel(x, lut_table, mapping):
    out, _ = kernel_with_results(x, lut_table, mapping)
    return out


if __name__ == "__main__":
    rng = np.random.default_rng(0)
    x = rng.random((B_FULL, IN), dtype=np.float32)
    lut = rng.standard_normal((OUT, 64), dtype=np.float32)
    mp = rng.integers(0, IN, (OUT, NB), dtype=np.int32)
    out = kernel(x, lut, mp)
    print(out.shape, out.dtype)


# revision 25
# speedup vs baseline: 1.2256x; 1.0304x over previous
"""Trainium2 Bass kernel for nn_BaseLUTLayer (soft-LUT layer), node-sharded.

Math: out[b,o] = sum_k lut[o,k] * prod_j (bit_j(k) ? x[b,m(o,j)] : 1-x[b,m(o,j)])

Walsh/multilinear form: with u = 2x-1 and ahat = lut @ W / 64 (host-side
Walsh transform over the 6-bit index), out = sum_S ahat_S * prod_{j in S} u_j.
Evaluated as a 6-level multilinear tree T_j[s] = T_{j+1}[s] + u_j*T_{j+1}[s+2^j].
The (static per run) mapping gather runs on HOST; the device streams
pre-gathered u rows via plain DMA.

Per core (node-sharded 8 ways): nodes [256c, 256(c+1)) as 2 chunks of 128
nodes-on-partitions. L1 (fused MACs vs per-node ahat scalars) + L2 + L3-mul
run once per chunk at full 1024 batch, IN PLACE in the t1 buffer:
    t1[16:32] *= u4 ; t1[0:16] += t1[16:32]   (L2)
    t1[8:16]  *= u3                            (L3 mul)
From L3-add down, work splits into 2 batch halves of 512: t3-hi + pr4 on
DVE in SBUF (bf16), then t3-lo/L4 accumulate in PSUM on PE (12 matmuls per
half into 4 banks, double-buffered across halves); pn2/pn1 muls on DVE read
PSUM (fp32 1x) between PE accumulate steps. The last half runs an all-DVE
tail so the kernel does not end on a PE ping-pong. L1 is split DVE
tensor_scalar (4x mode) / ScalarE activations by NTS; PSUM is drained by a
ScalarE copy, then DMA'd out.
"""

import numpy as np
import ml_dtypes

import concourse.bass as bass
import concourse.mybir as mybir
from concourse import bacc
from concourse import tile
from concourse.masks import make_identity
from concourse.bass_utils import run_bass_kernel_spmd

P = 128
IN = 1024
OUT = 2048
NB = 6
B_FULL = 1024
N_CORES = 8
NODES_PER_CORE = OUT // N_CORES  # 256
NCHUNK = NODES_PER_CORE // P     # 2
NHALF = 2
BH = B_FULL // NHALF             # 512
F32 = mybir.dt.float32
BF16 = mybir.dt.bfloat16

# DVE tensor_scalar L1 slices per chunk (rest go to ScalarE activations).
NTS = [14, 9]


def _mult():
    return mybir.AluOpType.mult


def _add():
    return mybir.AluOpType.add


def build_program():
    nc = bacc.Bacc("TRN2", target_bir_lowering=False, debug=False)

    gz = nc.dram_tensor("gz", [P, NCHUNK, NB, B_FULL], BF16, kind="ExternalInput").ap()
    lutg = nc.dram_tensor("lutg", [P, NCHUNK, 64], F32, kind="ExternalInput").ap()
    outs = nc.dram_tensor("outs", [P, NCHUNK, NHALF, BH], F32, kind="ExternalOutput").ap()

    with tile.TileContext(nc) as tc:
        with (
            tc.tile_pool(name="consts", bufs=1) as consts,
            tc.tile_pool(name="zpool", bufs=2) as zpool,
            tc.tile_pool(name="t1pool", bufs=2) as t1pool,
            tc.tile_pool(name="spool", bufs=2) as spool,
            tc.tile_pool(name="opool", bufs=2) as opool,
            tc.tile_pool(name="psum", bufs=2, space="PSUM") as psum,
        ):
            lutg_sb = consts.tile([P, NCHUNK, 64], F32)
            nc.sync.dma_start(lutg_sb, lutg)
            ident = consts.tile([P, P], BF16)
            make_identity(nc, ident)

            zs = {}
            t1s = {}
            state = {}

            def gather(c, part=None):
                # plain DMA of host-pre-gathered u rows (2KB each)
                if part is None or part == 0:
                    z = zpool.tile([P, NB, B_FULL], BF16, tag="z")
                    zs[c] = z
                z = zs[c]
                if part is None:
                    s0, s1 = 0, NB
                else:
                    s0, s1 = [(0, 1), (1, 3), (3, NB)][part]
                nc.sync.dma_start(z[:, s0:s1, :], gz[:, c, s0:s1, :])

            def u(c, s):
                return zs[c][:, s, :]  # [P, 1024]

            def uh(c, s, h):
                return zs[c][:, s, h * BH : (h + 1) * BH]

            def _splits(c):
                nts = NTS[c]
                dve_ks = list(range(16, 16 + min(nts, 16)))
                if nts > 16:
                    dve_ks += list(range(0, nts - 16))
                sc_hi = list(range(16 + min(nts, 16), 32))
                sc_lo = list(range(max(0, nts - 16), 16))
                sc_ks = sc_hi + sc_lo[8:][::-1] + sc_lo[:8][::-1]
                return dve_ks, sc_ks

            def scalar_l1(c):
                t1 = t1pool.tile([P, 32, B_FULL], BF16, tag="t1")
                t1s[c] = t1
                r5 = u(c, 0)
                _, sc_ks = _splits(c)
                for k in sc_ks:
                    nc.scalar.activation(
                        t1[:, k, :],
                        r5,
                        mybir.ActivationFunctionType.Identity,
                        bias=lutg_sb[:, c, k : k + 1],
                        scale=lutg_sb[:, c, 32 + k : 33 + k],
                    )

            def dve_l1(c):
                t1 = t1s[c]
                r5 = u(c, 0)
                dve_ks, _ = _splits(c)
                for k in dve_ks:
                    nc.vector.tensor_scalar(
                        out=t1[:, k, :],
                        in0=r5,
                        scalar1=lutg_sb[:, c, 32 + k : 33 + k],
                        scalar2=lutg_sb[:, c, k : k + 1],
                        op0=_mult(),
                        op1=_add(),
                    )

            def dve_l23(c):
                # in-place L2 + L3 mul at full 1024 width inside t1
                t1 = t1s[c]
                nc.vector.tensor_mul(
                    t1[:, 16:32, :],
                    u(c, 1)[:, None, :].broadcast_to([P, 16, B_FULL]),
                    t1[:, 16:32, :],
                )
                nc.vector.tensor_add(
                    t1[:, 8:16, :].rearrange("p a b -> p (a b)"),
                    t1[:, 24:32, :].rearrange("p a b -> p (a b)"),
                    t1[:, 8:16, :].rearrange("p a b -> p (a b)"),
                )
                nc.vector.tensor_add(
                    t1[:, 0:8, :].rearrange("p a b -> p (a b)"),
                    t1[:, 16:24, :].rearrange("p a b -> p (a b)"),
                    t1[:, 0:8, :].rearrange("p a b -> p (a b)"),
                )
                nc.vector.tensor_mul(
                    t1[:, 8:16, :],
                    u(c, 2)[:, None, :].broadcast_to([P, 8, B_FULL]),
                    t1[:, 8:16, :],
                )

            def dve_half_pe(c, h):
                # From L3-add down on one batch half; phase A: t3-hi/pr4 on
                # DVE, t3-lo + pr4 accumulation on PE (half_b finishes L5/L6).
                t1 = t1s[c]
                th = t1[:, :, h * BH : (h + 1) * BH]
                # t3 hi half on DVE in SBUF (bf16); pr4 = u2 * t3[4:8]
                t3h = spool.tile([P, 4, BH], BF16, tag="t3h")
                nc.vector.tensor_add(t3h[:], th[:, 4:8, :], th[:, 12:16, :])
                pr4 = spool.tile([P, 4, BH], BF16, tag="pr4")
                nc.vector.tensor_mul(
                    pr4,
                    uh(c, 3, h)[:, None, :].broadcast_to([P, 4, BH]),
                    t3h[:],
                )
                if c == NCHUNK - 1 and h == NHALF - 1:
                    # last half: all-DVE tail through t4 (rest in tail_b)
                    t3l = spool.tile([P, 4, BH], BF16, tag="t3l")
                    nc.vector.tensor_add(t3l[:], th[:, 0:4, :], th[:, 8:12, :])
                    t4 = spool.tile([P, 4, BH], BF16, tag="t4d")
                    nc.vector.tensor_add(
                        t4[:].rearrange("p a b -> p (a b)"),
                        t3l[:].rearrange("p a b -> p (a b)"),
                        pr4[:].rearrange("p a b -> p (a b)"),
                    )
                    state[("t4", c, h)] = t4
                    return
                acc = psum.tile([P, 4 * BH], F32, tag="accA")
                accv = acc[:].rearrange("p (a b) -> p a b", b=BH)
                for q in range(3, -1, -1):
                    sl = slice(q * BH, (q + 1) * BH)
                    nc.tensor.matmul(
                        acc[:, sl], ident, th[:, q, :], start=True, stop=False
                    )
                    nc.tensor.matmul(
                        acc[:, sl], ident, th[:, q + 8, :], start=False, stop=False
                    )
                    nc.tensor.matmul(
                        acc[:, sl], ident, pr4[:, q, :], start=False, stop=(q >= 2)
                    )
                state[("acc", c, h)] = acc

            def half_b(c, h):
                # L5/L6: pn2/pn1 muls (PSUM reads) + PE accumulate
                acc = state[("acc", c, h)]
                accv = acc[:].rearrange("p (a b) -> p a b", b=BH)
                pn2 = spool.tile([P, 2, BH], BF16, tag="pn2")
                nc.vector.tensor_mul(
                    pn2,
                    uh(c, 4, h)[:, None, :].broadcast_to([P, 2, BH]),
                    accv[:, 2:4, :],
                )
                nc.tensor.matmul(
                    acc[:, BH : 2 * BH], ident, pn2[:, 1, :], start=False, stop=True
                )
                nc.tensor.matmul(
                    acc[:, 0:BH], ident, pn2[:, 0, :], start=False, stop=False
                )
                pn1 = spool.tile([P, 1, BH], BF16, tag="pn1")
                nc.vector.tensor_mul(
                    pn1,
                    uh(c, 5, h)[:, None, :].broadcast_to([P, 1, BH]),
                    accv[:, 1:2, :],
                )
                nc.tensor.matmul(
                    acc[:, 0:BH], ident, pn1[:, 0, :], start=False, stop=True
                )

            def tail_b(c, h):
                t4 = state.pop(("t4", c, h))
                pn2 = spool.tile([P, 2, BH], BF16, tag="pn2")
                nc.vector.tensor_mul(
                    pn2,
                    uh(c, 4, h)[:, None, :].broadcast_to([P, 2, BH]),
                    t4[:, 2:4, :],
                )
                t5 = spool.tile([P, 2, BH], BF16, tag="t5d")
                nc.vector.tensor_add(t5, pn2, t4[:, 0:2, :])
                pn1 = spool.tile([P, 1, BH], BF16, tag="pn1")
                nc.vector.tensor_mul(
                    pn1,
                    uh(c, 5, h)[:, None, :].broadcast_to([P, 1, BH]),
                    t5[:, 1:2, :],
                )
                t6 = opool.tile([P, BH], F32, tag="t6d")
                nc.vector.tensor_add(t6, pn1[:, 0, :], t5[:, 0, :])
                state[("t6", c, h)] = t6

            def final(c, h):
                if ("t6", c, h) in state:
                    nc.sync.dma_start(outs[:, c, h, :], state.pop(("t6", c, h))[:])
                    return
                acc = state.pop(("acc", c, h))
                ot = opool.tile([P, BH], F32, tag="ot")
                nc.scalar.copy(ot[:], acc[:, 0:BH])
                nc.sync.dma_start(outs[:, c, h, :], ot[:])

            # ---- schedule ----
            gather(0, part=0)
            gather(0, part=1)
            gather(0, part=2)
            gather(1)
            scalar_l1(0)
            dve_l1(0)
            dve_l23(0)
            scalar_l1(1)
            dve_half_pe(0, 0)
            dve_l1(1)
            half_b(0, 0)
            final(0, 0)
            dve_half_pe(0, 1)
            dve_l23(1)
            half_b(0, 1)
            final(0, 1)
            dve_half_pe(1, 0)
            dve_half_pe(1, 1)
            tail_b(1, 1)
            half_b(1, 0)
            final(1, 0)
            final(1, 1)

    nc.compile()
    return nc


_CACHE: dict = {}


def _program():
    if "nc" not in _CACHE:
        _CACHE["nc"] = build_program()
    return _CACHE["nc"]


def _walsh_matrix():
    k = np.arange(64)
    bits = ((k[:, None] >> np.arange(6)[None, :]) & 1) * 2 - 1  # [64, 6]
    s = np.arange(64)
    smask = ((s[:, None] >> np.arange(6)[None, :]) & 1).astype(bool)
    W = np.ones((64, 64), dtype=np.float64)
    for j in range(6):
        W *= np.where(smask[None, :, j], bits[:, None, j], 1)
    return W


def make_inputs(x, lut_table, mapping):
    x = np.ascontiguousarray(x, dtype=np.float32)
    lut_table = np.ascontiguousarray(lut_table, dtype=np.float32)
    mapping = np.asarray(mapping)

    uT = (2.0 * x.T - 1.0).astype(ml_dtypes.bfloat16)  # [i, b]
    ahat = (lut_table.astype(np.float64) @ _walsh_matrix() / 64.0).astype(np.float32)

    in_maps = []
    for core in range(N_CORES):
        mp = mapping[core * NODES_PER_CORE : (core + 1) * NODES_PER_CORE]
        mp3 = mp.reshape(NCHUNK, P, NB)
        # gz[p, c, s, b] = u[b, mp3[c, p, 5-s]]  (slot s = wire 5-s)
        idx = mp3[:, :, ::-1].transpose(1, 0, 2)  # [P, NCHUNK, NB]
        gz_arr = np.ascontiguousarray(uT[idx])    # [P, NCHUNK, NB, B_FULL]

        lut3 = ahat[core * NODES_PER_CORE : (core + 1) * NODES_PER_CORE]
        lutg_arr = np.ascontiguousarray(
            lut3.reshape(NCHUNK, P, 64).transpose(1, 0, 2)
        )

        in_maps.append({"gz": gz_arr, "lutg": lutg_arr})
    return in_maps


def assemble_output(results):
    out = np.empty((B_FULL, OUT), dtype=np.float32)
    for core in range(N_CORES):
        arr = results[core]["outs"]  # [o_p, c, h, b']
        blk = arr.transpose(2, 3, 1, 0).reshape(B_FULL, NODES_PER_CORE)
        out[:, core * NODES_PER_CORE : (core + 1) * NODES_PER_CORE] = blk
    return out


def kernel_with_results(x, lut_table, mapping, **kwargs):
    nc = _program()
    in_maps = make_inputs(x, lut_table, mapping)
    res = run_bass_kernel_spmd(nc, in_maps, core_ids=list(range(N_CORES)), **kwargs)
    return assemble_output(res.results), res


def kernel(x, lut_table, mapping):
    out, _ = kernel_with_results(x, lut_table, mapping)
    return out


if __name__ == "__main__":
    rng = np.random.default_rng(0)
    x = rng.random((B_FULL, IN), dtype=np.float32)
    lut = rng.standard_normal((OUT, 64), dtype=np.float32)
    mp = rng.integers(0, IN, (OUT, NB), dtype=np.int32)
    out = kernel(x, lut, mp)
    print(out.shape, out.dtype)


# revision 26
# speedup vs baseline: 1.2463x; 1.0169x over previous
"""Trainium2 Bass kernel for nn_BaseLUTLayer (soft-LUT layer), node-sharded.

Math: out[b,o] = sum_k lut[o,k] * prod_j (bit_j(k) ? x[b,m(o,j)] : 1-x[b,m(o,j)])

Walsh/multilinear form: with u = 2x-1 and ahat = lut @ W / 64 (host-side
Walsh transform over the 6-bit index), out = sum_S ahat_S * prod_{j in S} u_j.
Evaluated as a 6-level multilinear tree T_j[s] = T_{j+1}[s] + u_j*T_{j+1}[s+2^j].
The (static per run) mapping gather runs on HOST; the device streams
pre-gathered u rows via plain DMA.

Per core (node-sharded 8 ways): nodes [256c, 256(c+1)) as 2 chunks of 128
nodes-on-partitions. L1 (fused MACs vs per-node ahat scalars) + L2 + L3-mul
run once per chunk at full 1024 batch, IN PLACE in the t1 buffer:
    t1[16:32] *= u4 ; t1[0:16] += t1[16:32]   (L2)
    t1[8:16]  *= u3                            (L3 mul)
From L3-add down, work splits into 2 batch halves of 512: t3-hi + pr4 on
DVE in SBUF (bf16), then t3-lo/L4 accumulate in PSUM on PE (12 matmuls per
half into 4 banks, double-buffered across halves); pn2/pn1 muls on DVE read
PSUM (fp32 1x) between PE accumulate steps. The last half runs an all-DVE
tail so the kernel does not end on a PE ping-pong. L1 is split DVE
tensor_scalar (4x mode) / ScalarE activations by NTS; PSUM is drained by a
ScalarE copy, then DMA'd out.
"""

import numpy as np
import ml_dtypes

import concourse.bass as bass
import concourse.mybir as mybir
from concourse import bacc
from concourse import tile
from concourse.masks import make_identity
from concourse.bass_utils import run_bass_kernel_spmd

P = 128
IN = 1024
OUT = 2048
NB = 6
B_FULL = 1024
N_CORES = 8
NODES_PER_CORE = OUT // N_CORES  # 256
NCHUNK = NODES_PER_CORE // P     # 2
NHALF = 2
BH = B_FULL // NHALF             # 512
F32 = mybir.dt.float32
BF16 = mybir.dt.bfloat16

# DVE tensor_scalar L1 slices per chunk (rest go to ScalarE activations).
NTS = [14, 9]


def _mult():
    return mybir.AluOpType.mult


def _add():
    return mybir.AluOpType.add


def build_program():
    nc = bacc.Bacc("TRN2", target_bir_lowering=False, debug=False)

    gz = nc.dram_tensor("gz", [P, NCHUNK, NB, B_FULL], BF16, kind="ExternalInput").ap()
    lutg = nc.dram_tensor("lutg", [P, NCHUNK, 64], F32, kind="ExternalInput").ap()
    outs = nc.dram_tensor("outs", [P, NCHUNK, NHALF, BH], F32, kind="ExternalOutput").ap()

    with tile.TileContext(nc) as tc:
        with (
            tc.tile_pool(name="consts", bufs=1) as consts,
            tc.tile_pool(name="zpool", bufs=2) as zpool,
            tc.tile_pool(name="t1pool", bufs=2) as t1pool,
            tc.tile_pool(name="spool", bufs=2) as spool,
            tc.tile_pool(name="opool", bufs=2) as opool,
            tc.tile_pool(name="psum", bufs=2, space="PSUM") as psum,
        ):
            lutg_sb = consts.tile([P, NCHUNK, 64], F32)
            nc.sync.dma_start(lutg_sb, lutg)
            ident = consts.tile([P, P], BF16)
            make_identity(nc, ident)

            zs = {}
            t1s = {}
            state = {}

            def gather(c, part=None):
                # plain DMA of host-pre-gathered u rows (2KB each)
                if part is None or part == 0:
                    z = zpool.tile([P, NB, B_FULL], BF16, tag="z")
                    zs[c] = z
                z = zs[c]
                if part is None:
                    s0, s1 = 0, NB
                else:
                    s0, s1 = [(0, 1), (1, 3), (3, NB)][part]
                nc.sync.dma_start(z[:, s0:s1, :], gz[:, c, s0:s1, :])

            def u(c, s):
                return zs[c][:, s, :]  # [P, 1024]

            def uh(c, s, h):
                return zs[c][:, s, h * BH : (h + 1) * BH]

            def _splits(c):
                nts = NTS[c]
                dve_ks = list(range(16, 16 + min(nts, 16)))
                if nts > 16:
                    dve_ks += list(range(0, nts - 16))
                sc_hi = list(range(16 + min(nts, 16), 32))
                sc_lo = list(range(max(0, nts - 16), 16))
                sc_ks = sc_hi + sc_lo[8:][::-1] + sc_lo[:8][::-1]
                return dve_ks, sc_ks

            def scalar_l1(c):
                t1 = t1pool.tile([P, 32, B_FULL], BF16, tag="t1")
                t1s[c] = t1
                r5 = u(c, 0)
                _, sc_ks = _splits(c)
                for k in sc_ks:
                    nc.scalar.activation(
                        t1[:, k, :],
                        r5,
                        mybir.ActivationFunctionType.Identity,
                        bias=lutg_sb[:, c, k : k + 1],
                        scale=lutg_sb[:, c, 32 + k : 33 + k],
                    )

            def dve_l1(c):
                t1 = t1s[c]
                r5 = u(c, 0)
                dve_ks, _ = _splits(c)
                for k in dve_ks:
                    nc.vector.tensor_scalar(
                        out=t1[:, k, :],
                        in0=r5,
                        scalar1=lutg_sb[:, c, 32 + k : 33 + k],
                        scalar2=lutg_sb[:, c, k : k + 1],
                        op0=_mult(),
                        op1=_add(),
                    )

            def dve_l23(c):
                # in-place L2 + L3 mul at full 1024 width inside t1
                t1 = t1s[c]
                nc.vector.tensor_mul(
                    t1[:, 16:32, :],
                    u(c, 1)[:, None, :].broadcast_to([P, 16, B_FULL]),
                    t1[:, 16:32, :],
                )
                nc.vector.tensor_add(
                    t1[:, 8:16, :].rearrange("p a b -> p (a b)"),
                    t1[:, 24:32, :].rearrange("p a b -> p (a b)"),
                    t1[:, 8:16, :].rearrange("p a b -> p (a b)"),
                )
                nc.vector.tensor_add(
                    t1[:, 0:8, :].rearrange("p a b -> p (a b)"),
                    t1[:, 16:24, :].rearrange("p a b -> p (a b)"),
                    t1[:, 0:8, :].rearrange("p a b -> p (a b)"),
                )
                nc.vector.tensor_mul(
                    t1[:, 8:16, :],
                    u(c, 2)[:, None, :].broadcast_to([P, 8, B_FULL]),
                    t1[:, 8:16, :],
                )

            def dve_half_pe(c, h):
                # From L3-add down on one batch half; phase A: t3-hi/pr4 on
                # DVE, t3-lo + pr4 accumulation on PE (half_b finishes L5/L6).
                t1 = t1s[c]
                th = t1[:, :, h * BH : (h + 1) * BH]
                # t3 hi half on DVE in SBUF (bf16); pr4 = u2 * t3[4:8]
                t3h = spool.tile([P, 4, BH], BF16, tag="t3h")
                nc.vector.tensor_add(t3h[:], th[:, 4:8, :], th[:, 12:16, :])
                pr4 = spool.tile([P, 4, BH], BF16, tag="pr4")
                nc.vector.tensor_mul(
                    pr4,
                    uh(c, 3, h)[:, None, :].broadcast_to([P, 4, BH]),
                    t3h[:],
                )
                if c == NCHUNK - 1 and h == NHALF - 1:
                    # last half: all-DVE tail through t4 (rest in tail_b)
                    t3l = spool.tile([P, 4, BH], BF16, tag="t3l")
                    nc.vector.tensor_add(t3l[:], th[:, 0:4, :], th[:, 8:12, :])
                    t4 = spool.tile([P, 4, BH], BF16, tag="t4d")
                    nc.vector.tensor_add(
                        t4[:].rearrange("p a b -> p (a b)"),
                        t3l[:].rearrange("p a b -> p (a b)"),
                        pr4[:].rearrange("p a b -> p (a b)"),
                    )
                    state[("t4", c, h)] = t4
                    return
                acc = psum.tile([P, 4 * BH], F32, tag="accA")
                accv = acc[:].rearrange("p (a b) -> p a b", b=BH)
                # th-pair matmuls first: they don't need pr4, so PE can
                # start as soon as L23 lands while DVE computes t3h/pr4
                for q in range(3, -1, -1):
                    sl = slice(q * BH, (q + 1) * BH)
                    nc.tensor.matmul(
                        acc[:, sl], ident, th[:, q, :], start=True, stop=False
                    )
                    nc.tensor.matmul(
                        acc[:, sl], ident, th[:, q + 8, :], start=False, stop=False
                    )
                for q in range(3, -1, -1):
                    sl = slice(q * BH, (q + 1) * BH)
                    nc.tensor.matmul(
                        acc[:, sl], ident, pr4[:, q, :], start=False, stop=(q >= 2)
                    )
                state[("acc", c, h)] = acc

            def half_b(c, h):
                # L5/L6: pn2/pn1 muls (PSUM reads) + PE accumulate
                acc = state[("acc", c, h)]
                accv = acc[:].rearrange("p (a b) -> p a b", b=BH)
                pn2 = spool.tile([P, 2, BH], BF16, tag="pn2")
                nc.vector.tensor_mul(
                    pn2,
                    uh(c, 4, h)[:, None, :].broadcast_to([P, 2, BH]),
                    accv[:, 2:4, :],
                )
                nc.tensor.matmul(
                    acc[:, BH : 2 * BH], ident, pn2[:, 1, :], start=False, stop=True
                )
                nc.tensor.matmul(
                    acc[:, 0:BH], ident, pn2[:, 0, :], start=False, stop=False
                )
                pn1 = spool.tile([P, 1, BH], BF16, tag="pn1")
                nc.vector.tensor_mul(
                    pn1,
                    uh(c, 5, h)[:, None, :].broadcast_to([P, 1, BH]),
                    accv[:, 1:2, :],
                )
                nc.tensor.matmul(
                    acc[:, 0:BH], ident, pn1[:, 0, :], start=False, stop=True
                )

            def tail_b(c, h):
                t4 = state.pop(("t4", c, h))
                pn2 = spool.tile([P, 2, BH], BF16, tag="pn2")
                nc.vector.tensor_mul(
                    pn2,
                    uh(c, 4, h)[:, None, :].broadcast_to([P, 2, BH]),
                    t4[:, 2:4, :],
                )
                t5 = spool.tile([P, 2, BH], BF16, tag="t5d")
                nc.vector.tensor_add(t5, pn2, t4[:, 0:2, :])
                pn1 = spool.tile([P, 1, BH], BF16, tag="pn1")
                nc.vector.tensor_mul(
                    pn1,
                    uh(c, 5, h)[:, None, :].broadcast_to([P, 1, BH]),
                    t5[:, 1:2, :],
                )
                t6 = opool.tile([P, BH], F32, tag="t6d")
                nc.vector.tensor_add(t6, pn1[:, 0, :], t5[:, 0, :])
                state[("t6", c, h)] = t6

            def final(c, h):
                if ("t6", c, h) in state:
                    nc.sync.dma_start(outs[:, c, h, :], state.pop(("t6", c, h))[:])
                    return
                acc = state.pop(("acc", c, h))
                ot = opool.tile([P, BH], F32, tag="ot")
                nc.scalar.copy(ot[:], acc[:, 0:BH])
                nc.sync.dma_start(outs[:, c, h, :], ot[:])

            # ---- schedule ----
            gather(0, part=0)
            gather(0, part=1)
            gather(0, part=2)
            gather(1)
            scalar_l1(0)
            dve_l1(0)
            dve_l23(0)
            scalar_l1(1)
            dve_half_pe(0, 0)
            dve_l1(1)
            half_b(0, 0)
            final(0, 0)
            dve_half_pe(0, 1)
            dve_l23(1)
            half_b(0, 1)
            final(0, 1)
            dve_half_pe(1, 0)
            dve_half_pe(1, 1)
            tail_b(1, 1)
            half_b(1, 0)
            final(1, 0)
            final(1, 1)

    nc.compile()
    return nc


_CACHE: dict = {}


def _program():
    if "nc" not in _CACHE:
        _CACHE["nc"] = build_program()
    return _CACHE["nc"]


def _walsh_matrix():
    k = np.arange(64)
    bits = ((k[:, None] >> np.arange(6)[None, :]) & 1) * 2 - 1  # [64, 6]
    s = np.arange(64)
    smask = ((s[:, None] >> np.arange(6)[None, :]) & 1).astype(bool)
    W = np.ones((64, 64), dtype=np.float64)
    for j in range(6):
        W *= np.where(smask[None, :, j], bits[:, None, j], 1)
    return W


def make_inputs(x, lut_table, mapping):
    x = np.ascontiguousarray(x, dtype=np.float32)
    lut_table = np.ascontiguousarray(lut_table, dtype=np.float32)
    mapping = np.asarray(mapping)

    uT = (2.0 * x.T - 1.0).astype(ml_dtypes.bfloat16)  # [i, b]
    ahat = (lut_table.astype(np.float64) @ _walsh_matrix() / 64.0).astype(np.float32)

    in_maps = []
    for core in range(N_CORES):
        mp = mapping[core * NODES_PER_CORE : (core + 1) * NODES_PER_CORE]
        mp3 = mp.reshape(NCHUNK, P, NB)
        # gz[p, c, s, b] = u[b, mp3[c, p, 5-s]]  (slot s = wire 5-s)
        idx = mp3[:, :, ::-1].transpose(1, 0, 2)  # [P, NCHUNK, NB]
        gz_arr = np.ascontiguousarray(uT[idx])    # [P, NCHUNK, NB, B_FULL]

        lut3 = ahat[core * NODES_PER_CORE : (core + 1) * NODES_PER_CORE]
        lutg_arr = np.ascontiguousarray(
            lut3.reshape(NCHUNK, P, 64).transpose(1, 0, 2)
        )

        in_maps.append({"gz": gz_arr, "lutg": lutg_arr})
    return in_maps


def assemble_output(results):
    out = np.empty((B_FULL, OUT), dtype=np.float32)
    for core in range(N_CORES):
        arr = results[core]["outs"]  # [o_p, c, h, b']
        blk = arr.transpose(2, 3, 1, 0).reshape(B_FULL, NODES_PER_CORE)
        out[:, core * NODES_PER_CORE : (core + 1) * NODES_PER_CORE] = blk
    return out


def kernel_with_results(x, lut_table, mapping, **kwargs):
    nc = _program()
    in_maps = make_inputs(x, lut_table, mapping)
    res = run_bass_kernel_spmd(nc, in_maps, core_ids=list(range(N_CORES)), **kwargs)
    return assemble_output(res.results), res


def kernel(x, lut_table, mapping):
    out, _ = kernel_with_results(x, lut_table, mapping)
    return out


if __name__ == "__main__":
    rng = np.random.default_rng(0)
    x = rng.random((B_FULL, IN), dtype=np.float32)
    lut = rng.standard_normal((OUT, 64), dtype=np.float32)
    mp = rng.integers(0, IN, (OUT, NB), dtype=np.int32)
    out = kernel(x, lut, mp)
    print(out.shape, out.dtype)
